# revision 33
# baseline (speedup 1.0000x reference)
"""Causal single-head attention  B=4, T=4096, C=1024, D=64  on 8 TRN2 cores.

Sharding: 2 cores per batch; queries split by 128-col chunk parity
(core p owns natural chunks {2u+p}) for causal balance. The parity is
handled purely by data: p=1 cores get xt with adjacent 128-col chunks
swapped, plus an all-ones "mask2" (p=0: all-zeros) for the one
program-order-future-but-naturally-past chunk per query chunk.

Per core (bf16 matmuls, fp32 accum), all in program chunk order:
  k|v proj  [seq,D]: out[kv,128] = xt_c.T @ [Wk|Wv]_c   (M=128/pass)
  q proj    [seq,D]: out[q,64]   = xt_g.T @ Wq_c        (M=64/pass)
  kT,qT via PE transpose (D=64 rows needed on partitions for scores)
  scoresT   = kT_chunk.T @ qT    (kv on partitions, q free)
  probs     = exp(0.125 * scoresT)  one ACT op per chunk-PAIR
              (two adjacent PSUM banks, right/left aligned)
  masks     : diag chunk = shared lower-tri; next chunk = mask2 (0/1)
  AV flipped: out[q,65] += probsT_slice.T @ [v|1]       (M=65/pass)
  host: out[b, owned chunks] = (acc[:, :64] / acc[:, 64:65])
"""

import sys

sys.path.insert(0, "/opt/trn_rl_repo")

import numpy as np
import ml_dtypes

B, T, C, D = 4, 4096, 1024, 64
NCC = C // 128          # 8 contraction chunks
NKV = T // 128          # 32 kv chunks (program order)
NT = 4                  # q tiles (4 q-chunks each, 512 cols)
N_CORES = 8

_compiled = None


def _build_nc(loop_n=None):
    import contextlib
    import concourse.bass as bass
    import concourse.bacc as bacc
    import concourse.mybir as mybir
    from concourse.tile import TileContext
    from concourse.masks import make_identity

    f32 = mybir.dt.float32
    bf16 = mybir.dt.bfloat16

    nc = bacc.Bacc("TRN2", target_bir_lowering=False, debug=False)
    xt = nc.dram_tensor("xt", (C, T), bf16, kind="ExternalInput")
    wq = nc.dram_tensor("wq", (128, NCC * D), bf16, kind="ExternalInput")
    wkv = nc.dram_tensor("wkv", (128, NCC * 2 * D), bf16, kind="ExternalInput")
    masks = nc.dram_tensor("masks", (128, 256), bf16, kind="ExternalInput")
    out = nc.dram_tensor("out", (T // 2, D + 1), f32, kind="ExternalOutput")

    with TileContext(nc) as tc:
        with (
            tc.tile_pool(name="const", bufs=1) as constp,
            tc.tile_pool(name="xtf", bufs=24) as xtfp,
            tc.tile_pool(name="xtb", bufs=16) as xtbp,
            tc.tile_pool(name="kvg", bufs=8) as kvgp,
            tc.tile_pool(name="ktg", bufs=8) as ktgp,
            tc.tile_pool(name="qsb", bufs=2) as qsbp,
            tc.tile_pool(name="qtg", bufs=2) as qtgp,
            tc.tile_pool(name="probs", bufs=4) as probsp,
            tc.tile_pool(name="osb", bufs=4) as osbp,
            tc.tile_pool(name="ps_s", bufs=2, space="PSUM") as ps_sp,
            tc.tile_pool(name="ps_kv", bufs=2, space="PSUM") as ps_kvp,
            tc.tile_pool(name="ps_t", bufs=1, space="PSUM") as ps_tp,
            tc.tile_pool(name="ps_av", bufs=1, space="PSUM") as ps_avp,
        ):
            ident = constp.tile([128, 128], bf16, tag="ident")
            make_identity(nc, ident)

            wq_sb = constp.tile([128, NCC * D], bf16, tag="wq")
            wkv_sb = constp.tile([128, NCC * 2 * D], bf16, tag="wkv")
            mask_sb = constp.tile([128, 256], bf16, tag="masks")
            # wkv gates the first projection: it goes first on sync; wq
            # and masks are needed later and are issued after the early
            # x tiles (see the DMA schedule below)
            nc.sync.dma_start(out=wkv_sb, in_=wkv[:, :])

            loop_cm = (
                tc.For_i(0, loop_n, 1) if loop_n else contextlib.nullcontext()
            )
            with loop_cm:
              xtiles = {}      # (cc, kind, j) -> tile
              kvg = {}         # kv group gi -> [128, 4, 130] bf16
              ktg = {}         # kv group gi -> [64, 512] bf16
              qt_tiles = {}    # tau -> [64, 512] bf16

              # xt tiling: fine [128,512] for cols 0:1024 and 3584:4096,
              # big [128,1280] for cols 1024:3584. Issue queues are round-
              # robined so no single sequencer bottlenecks DMA issue.
              def dma_xt(kind, j, cc, eng):
                  if kind == "f":
                      t = xtfp.tile([128, 512], bf16, tag="xf")
                      eng.dma_start(
                          out=t,
                          in_=xt[cc * 128:(cc + 1) * 128,
                                 j * 512:(j + 1) * 512],
                      )
                  else:
                      t = xtbp.tile([128, 1280], bf16, tag="xb")
                      eng.dma_start(
                          out=t,
                          in_=xt[cc * 128:(cc + 1) * 128,
                                 1024 + j * 1280:1024 + (j + 1) * 1280],
                      )
                  xtiles[(cc, kind, j)] = t

              def xcol(cc, c):
                  """AP for xt rows [128cc:+128], program cols [128c:+128)."""
                  col = c * 128
                  if col < 1024:
                      j = col // 512
                      return xtiles[(cc, "f", j)][:, col - 512 * j:
                                                  col - 512 * j + 128]
                  if col >= 3584:
                      return xtiles[(cc, "f", 7)][:, col - 3584:col - 3456]
                  g2 = (col - 1024) // 1280
                  o = col - 1024 - g2 * 1280
                  return xtiles[(cc, "b", g2)][:, o:o + 128]

              # ---- projection emission units -------------------------
              def proj_kv_chunk(gi, idx, ps):
                  c = gi * 4 + idx
                  for cc in range(NCC):
                      nc.tensor.matmul(
                          ps[:, idx, :],
                          lhsT=xcol(cc, c),
                          rhs=wkv_sb[:, cc * 128:(cc + 1) * 128],
                          start=(cc == 0),
                          stop=(cc == NCC - 1),
                      )

              def finish_kv(gi, ps):
                  g = kvgp.tile([128, 4, 130], bf16, tag="kvg")
                  nc.vector.tensor_copy(g[:, :, 0:128], ps)
                  nc.vector.memset(g[:, :, 128:129], 1.0)
                  kt_ps = ps_tp.tile([64, 512], bf16, tag="ktps")
                  for idx in range(4):
                      nc.tensor.transpose(
                          kt_ps[:, idx * 128:(idx + 1) * 128],
                          g[:, idx, 0:64],
                          ident,
                      )
                  kt = ktgp.tile([64, 512], bf16, tag="ktg")
                  nc.vector.tensor_copy(kt, kt_ps)
                  kvg[gi] = g
                  ktg[gi] = kt

              def proj_q(tau, r, ps):
                  g = 8 * tau + 2 * r
                  for cc in range(NCC):
                      nc.tensor.matmul(
                          ps[:, r, 0:64],
                          lhsT=xcol(cc, g),
                          rhs=wq_sb[:, cc * D:(cc + 1) * D],
                          start=(cc == 0),
                          stop=(cc == NCC - 1),
                      )

              def finish_q(tau, ps):
                  q_sb = qsbp.tile([128, 4, 64], bf16, tag="qsb")
                  nc.vector.tensor_copy(q_sb, ps[:, :, 0:64])
                  qt_ps = ps_tp.tile([64, 512], bf16, tag="ktps")
                  for r in range(4):
                      nc.tensor.transpose(
                          qt_ps[:, r * 128:(r + 1) * 128],
                          q_sb[:, r, :],
                          ident,
                      )
                  qt = qtgp.tile([64, 512], bf16, tag="qtg")
                  nc.vector.tensor_copy(qt, qt_ps)
                  qt_tiles[tau] = qt

              _pstate = {}

              def kv_units(gi):
                  """5 closures projecting kv group gi."""
                  def mk_kv(idx):
                      def f():
                          if idx == 0:
                              _pstate[gi] = ps_kvp.tile(
                                  [128, 4, 128], f32, tag="pskv",
                                  name="pskv",
                              )
                          proj_kv_chunk(gi, idx, _pstate[gi])
                      return f

                  return [mk_kv(i) for i in range(4)] + [
                      lambda: finish_kv(gi, _pstate.pop(gi))
                  ]

              def q_units(tau):
                  """5 closures projecting + transposing q tile tau."""
                  key = ("q", tau)

                  def mk_q(r):
                      def f():
                          if r == 0:
                              _pstate[key] = ps_kvp.tile(
                                  [128, 4, 128], f32, tag="pskv",
                                  name="psq",
                              )
                          proj_q(tau, r, _pstate[key])
                      return f

                  return [mk_q(r) for r in range(4)] + [
                      lambda: finish_q(tau, _pstate.pop(key))
                  ]

              # ---- attention ----------------------------------------
              def attend(tau, sched):
                  """A(tau), software-pipelined two pairs deep; `sched`
                  maps slot index -> closures (deadline-spread projection
                  work for this and the next tau, filling PE idle during
                  ACT-bound stretches)."""
                  qt = qt_tiles.pop(tau)
                  npairs = 4 * tau + 4
                  ps_av = ps_avp.tile([128, 4, 65], f32, tag="psav")
                  pend = {}

                  def emit_scores(pr):
                      c = 2 * pr
                      diag = pr >= 4 * tau
                      r0 = pr - 4 * tau if diag else 0
                      m = 512 - 128 * r0
                      ps = ps_sp.tile([128, 1024], f32, tag="pss")
                      # even chunk right-aligned bank0, odd left bank1
                      nc.tensor.matmul(
                          ps[:, 512 - m:512],
                          lhsT=ktg[c // 4][:, (c % 4) * 128:
                                           (c % 4 + 1) * 128],
                          rhs=qt[:, 128 * r0:512],
                          start=True, stop=True,
                      )
                      c1 = c + 1
                      nc.tensor.matmul(
                          ps[:, 512:512 + m],
                          lhsT=ktg[c1 // 4][:, (c1 % 4) * 128:
                                            (c1 % 4 + 1) * 128],
                          rhs=qt[:, 128 * r0:512],
                          start=True, stop=True,
                      )
                      p = probsp.tile([128, 1024], bf16, tag="p")
                      nc.scalar.activation(
                          p[:, 0:2 * m], ps[:, 512 - m:512 + m],
                          mybir.ActivationFunctionType.Exp, scale=0.125,
                      )
                      pend[pr] = (p, m, r0, diag)

                  def emit_av(pr):
                      p, m, r0, diag = pend.pop(pr)
                      if diag:
                          nc.vector.tensor_mul(
                              p[:, 0:128], p[:, 0:128], mask_sb[:, 0:128]
                          )
                          nc.vector.tensor_mul(
                              p[:, m:m + 128], p[:, m:m + 128],
                              mask_sb[:, 128:256],
                          )
                      for half in (0, 1):
                          ch = 2 * pr + half
                          off = half * m
                          vslice = kvg[ch // 4][:, ch % 4, 64:129]
                          for r in range(r0, 4):
                              nc.tensor.matmul(
                                  ps_av[:, r, :],
                                  lhsT=p[:, off + (r - r0) * 128:
                                         off + (r - r0 + 1) * 128],
                                  rhs=vslice,
                                  # one start per psum tile: the bank-wide
                                  # pending-zero mark must not be re-armed
                                  # by later region starts
                                  start=(ch == 0 and r == 0),
                                  stop=(ch == 8 * tau + 2 * r + 1),
                                  skip_group_check=True,
                              )

                  def emit_out(r):
                      # region r final after its diag pair: drain it now so
                      # the output DMA overlaps the remaining pairs
                      o_sb = osbp.tile([128, 65], f32, tag="osb")
                      nc.vector.tensor_copy(o_sb, ps_av[:, r, :])
                      eng = nc.scalar if tau < 2 else nc.sync
                      eng.dma_start(
                          out=out[512 * tau + 128 * r:
                                  512 * tau + 128 * (r + 1), :],
                          in_=o_sb,
                      )

                  # 2-deep software pipeline: scores of pairs i+1, i+2 are
                  # issued before AV of pair i so PE never heads-of-line
                  # blocks on the exp latency
                  depth = 3
                  for pr in range(npairs + depth):
                      for u in sched.get(pr, ()):
                          u()
                      if pr < npairs:
                          emit_scores(pr)
                      if pr >= depth:
                          emit_av(pr - depth)
                          if pr - depth >= 4 * tau:
                              emit_out(pr - depth - 4 * tau)
                  for u in sched.get("end", ()):
                      u()
                  del depth

              # ---- schedule -----------------------------------------
              # upfront DMA issue, need-ordered, spread over the three
              # DMA-capable queues (SP/ACT on HWDGE, gpsimd on SWDGE).
              # ACT gets only early ones (its SEQ issues exps from ~9us).
              rr = [nc.sync, nc.scalar, nc.gpsimd]
              for j in (0, 1):          # fine cols 0:1024
                  for cc in range(NCC):
                      dma_xt("f", j, cc, rr[(j * NCC + cc) % 3])
              nc.gpsimd.dma_start(out=wq_sb, in_=wq[:, :])
              nc.gpsimd.dma_start(out=mask_sb, in_=masks[:, :])
              for cc in range(NCC):     # big cols 1024:2304
                  # scalar (ACT) must be free before exps start (~8us):
                  # it gets only one; sync/gpsimd split the rest
                  eng = (nc.scalar if cc == 0
                         else nc.sync if cc < 5 else nc.gpsimd)
                  dma_xt("b", 0, cc, eng)
              for cc in range(NCC):     # big cols 2304:3584
                  dma_xt("b", 1, cc, nc.sync if cc % 2 else nc.gpsimd)
              for cc in range(NCC):     # fine cols 3584:4096
                  dma_xt("f", 7, cc, nc.sync if cc % 2 else nc.gpsimd)

              def spread(m, units, s0, s1):
                  # distribute units evenly over slots [s0, s1)
                  n = len(units)
                  span = max(1, s1 - s0)
                  for i, u in enumerate(units):
                      s = min(s1 - 1, s0 + (i * span) // n)
                      m.setdefault(s, []).append(u)

              # A(tau) pair pr touches kv chunks 2pr,2pr+1, i.e. kv group
              # g first at pair 2g; with the 2-deep pipeline its scores
              # are emitted at slot 2g. qt(tau) is needed at slot 0, so
              # q is projected during the previous tau.
              for u in kv_units(0) + q_units(0):
                  u()
              for tau in range(NT):
                  sched = {}
                  # slot 0 stays unit-free so each tau's first scores
                  # (and its exp) issue immediately at the boundary
                  if tau == 0:
                      spread(sched, kv_units(1), 1, 3)
                      spread(sched, q_units(1), 3, 6)
                  else:
                      spread(sched, kv_units(2 * tau), 1, 4 * tau)
                      spread(sched, kv_units(2 * tau + 1),
                             4 * tau, 4 * tau + 2)
                      if tau + 1 < NT:
                          spread(sched, q_units(tau + 1),
                                 4 * tau + 2, 4 * tau + 5)
                  attend(tau, sched)

    nc.compile()
    return nc


def _get_compiled():
    global _compiled
    if _compiled is None:
        _compiled = _build_nc()
    return _compiled


def _host_inputs(x, Wq, Wk, Wv):
    bf = ml_dtypes.bfloat16
    wq = np.concatenate(
        [Wq[c * 128:(c + 1) * 128] for c in range(NCC)], axis=1
    ).astype(bf)
    wkv_full = np.concatenate([Wk, Wv], axis=1)
    wkv = np.concatenate(
        [wkv_full[c * 128:(c + 1) * 128] for c in range(NCC)], axis=1
    ).astype(bf)

    j = np.arange(128)[:, None]
    i = np.arange(128)[None, :]
    tri = (j <= i).astype(np.float32)

    in_maps = []
    for core in range(N_CORES):
        b, p = core // 2, core % 2
        xb = x[b]                        # (T, C)
        if p == 1:
            xb = xb.reshape(NKV // 2, 2, 128, C)[:, ::-1].reshape(T, C)
        xtc = np.ascontiguousarray(xb.T, dtype=bf)
        m2 = np.full((128, 128), float(p), np.float32)
        mask = np.concatenate([tri, m2], axis=1).astype(bf)
        in_maps.append({"xt": xtc, "wq": wq, "wkv": wkv, "masks": mask})
    return in_maps


def kernel(x, Wq, Wk, Wv):
    from concourse.bass_utils import run_bass_kernel_spmd

    nc = _get_compiled()
    in_maps = _host_inputs(x, Wq, Wk, Wv)
    res = run_bass_kernel_spmd(nc, in_maps, core_ids=list(range(N_CORES)))

    out_full = np.empty((B, T, D), dtype=np.float32)
    for core in range(N_CORES):
        b, p = core // 2, core % 2
        acc = res.results[core]["out"]          # (2048, 65) f32
        o = acc[:, :D] / acc[:, D:D + 1]
        o = o.reshape(16, 128, D)
        for u in range(16):
            n = 2 * u + p
            out_full[b, 128 * n:128 * (n + 1), :] = o[u]
    return out_full


# revision 34
# speedup vs baseline: 1.0029x; 1.0029x over previous
"""Causal single-head attention  B=4, T=4096, C=1024, D=64  on 8 TRN2 cores.

Sharding: 2 cores per batch; queries split by 128-col chunk parity
(core p owns natural chunks {2u+p}) for causal balance. The parity is
handled purely by data: p=1 cores get xt with adjacent 128-col chunks
swapped, plus an all-ones "mask2" (p=0: all-zeros) for the one
program-order-future-but-naturally-past chunk per query chunk.

Per core (bf16 matmuls, fp32 accum), all in program chunk order:
  k|v proj  [seq,D]: out[kv,128] = xt_c.T @ [Wk|Wv]_c   (M=128/pass)
  q proj    [seq,D]: out[q,64]   = xt_g.T @ Wq_c        (M=64/pass)
  kT,qT via PE transpose (D=64 rows needed on partitions for scores)
  scoresT   = kT_chunk.T @ qT    (kv on partitions, q free)
  probs     = exp(0.125 * scoresT)  one ACT op per chunk-PAIR
              (two adjacent PSUM banks, right/left aligned)
  masks     : diag chunk = shared lower-tri; next chunk = mask2 (0/1)
  AV flipped: out[q,65] += probsT_slice.T @ [v|1]       (M=65/pass)
  host: out[b, owned chunks] = (acc[:, :64] / acc[:, 64:65])
"""

import sys

sys.path.insert(0, "/opt/trn_rl_repo")

import numpy as np
import ml_dtypes

B, T, C, D = 4, 4096, 1024, 64
NCC = C // 128          # 8 contraction chunks
NKV = T // 128          # 32 kv chunks (program order)
NT = 4                  # q tiles (4 q-chunks each, 512 cols)
N_CORES = 8

_compiled = None


def _build_nc(loop_n=None):
    import contextlib
    import concourse.bass as bass
    import concourse.bacc as bacc
    import concourse.mybir as mybir
    from concourse.tile import TileContext
    from concourse.masks import make_identity

    f32 = mybir.dt.float32
    bf16 = mybir.dt.bfloat16

    nc = bacc.Bacc("TRN2", target_bir_lowering=False, debug=False)
    xt = nc.dram_tensor("xt", (C, T), bf16, kind="ExternalInput")
    wq = nc.dram_tensor("wq", (128, NCC * D), bf16, kind="ExternalInput")
    wkv = nc.dram_tensor("wkv", (128, NCC * 2 * D), bf16, kind="ExternalInput")
    masks = nc.dram_tensor("masks", (128, 256), bf16, kind="ExternalInput")
    out = nc.dram_tensor("out", (T // 2, D + 1), f32, kind="ExternalOutput")

    with TileContext(nc) as tc:
        with (
            tc.tile_pool(name="const", bufs=1) as constp,
            tc.tile_pool(name="xtf", bufs=24) as xtfp,
            tc.tile_pool(name="xtb", bufs=16) as xtbp,
            tc.tile_pool(name="kvg", bufs=8) as kvgp,
            tc.tile_pool(name="ktg", bufs=8) as ktgp,
            tc.tile_pool(name="qsb", bufs=2) as qsbp,
            tc.tile_pool(name="qtg", bufs=2) as qtgp,
            tc.tile_pool(name="probs", bufs=4) as probsp,
            tc.tile_pool(name="osb", bufs=4) as osbp,
            tc.tile_pool(name="ps_s", bufs=2, space="PSUM") as ps_sp,
            tc.tile_pool(name="ps_kv", bufs=2, space="PSUM") as ps_kvp,
            tc.tile_pool(name="ps_t", bufs=1, space="PSUM") as ps_tp,
            tc.tile_pool(name="ps_av", bufs=1, space="PSUM") as ps_avp,
        ):
            ident = constp.tile([128, 128], bf16, tag="ident")
            make_identity(nc, ident)

            wq_sb = constp.tile([128, NCC * D], bf16, tag="wq")
            wkv_sb = constp.tile([128, NCC * 2 * D], bf16, tag="wkv")
            mask_sb = constp.tile([128, 256], bf16, tag="masks")
            # wkv gates the first projection: it goes first on sync; wq
            # and masks are needed later and are issued after the early
            # x tiles (see the DMA schedule below)
            nc.sync.dma_start(out=wkv_sb, in_=wkv[:, :])

            loop_cm = (
                tc.For_i(0, loop_n, 1) if loop_n else contextlib.nullcontext()
            )
            with loop_cm:
              xtiles = {}      # (cc, kind, j) -> tile
              kvg = {}         # kv group gi -> [128, 4, 130] bf16
              ktg = {}         # kv group gi -> [64, 512] bf16
              qt_tiles = {}    # tau -> [64, 512] bf16

              # xt tiling: fine [128,512] for cols 0:1024 and 3584:4096,
              # big [128,1280] for cols 1024:3584. Issue queues are round-
              # robined so no single sequencer bottlenecks DMA issue.
              def dma_xt(kind, j, cc, eng):
                  if kind == "f":
                      t = xtfp.tile([128, 512], bf16, tag="xf")
                      eng.dma_start(
                          out=t,
                          in_=xt[cc * 128:(cc + 1) * 128,
                                 j * 512:(j + 1) * 512],
                      )
                  else:
                      t = xtbp.tile([128, 1280], bf16, tag="xb")
                      eng.dma_start(
                          out=t,
                          in_=xt[cc * 128:(cc + 1) * 128,
                                 1024 + j * 1280:1024 + (j + 1) * 1280],
                      )
                  xtiles[(cc, kind, j)] = t

              def xcol(cc, c):
                  """AP for xt rows [128cc:+128], program cols [128c:+128)."""
                  col = c * 128
                  if col < 1024:
                      j = col // 512
                      return xtiles[(cc, "f", j)][:, col - 512 * j:
                                                  col - 512 * j + 128]
                  if col >= 3584:
                      return xtiles[(cc, "f", 7)][:, col - 3584:col - 3456]
                  g2 = (col - 1024) // 1280
                  o = col - 1024 - g2 * 1280
                  return xtiles[(cc, "b", g2)][:, o:o + 128]

              # ---- projection emission units -------------------------
              def proj_kv_chunk(gi, idx, ps):
                  c = gi * 4 + idx
                  for cc in range(NCC):
                      nc.tensor.matmul(
                          ps[:, idx, :],
                          lhsT=xcol(cc, c),
                          rhs=wkv_sb[:, cc * 128:(cc + 1) * 128],
                          start=(cc == 0),
                          stop=(cc == NCC - 1),
                      )

              def finish_kv(gi, ps):
                  g = kvgp.tile([128, 4, 130], bf16, tag="kvg")
                  nc.vector.tensor_copy(g[:, :, 0:128], ps)
                  nc.vector.memset(g[:, :, 128:129], 1.0)
                  kt_ps = ps_tp.tile([64, 512], bf16, tag="ktps")
                  for idx in range(4):
                      nc.tensor.transpose(
                          kt_ps[:, idx * 128:(idx + 1) * 128],
                          g[:, idx, 0:64],
                          ident,
                      )
                  kt = ktgp.tile([64, 512], bf16, tag="ktg")
                  nc.vector.tensor_copy(kt, kt_ps)
                  kvg[gi] = g
                  ktg[gi] = kt

              def proj_q(tau, r, ps):
                  g = 8 * tau + 2 * r
                  for cc in range(NCC):
                      nc.tensor.matmul(
                          ps[:, r, 0:64],
                          lhsT=xcol(cc, g),
                          rhs=wq_sb[:, cc * D:(cc + 1) * D],
                          start=(cc == 0),
                          stop=(cc == NCC - 1),
                      )

              def finish_q(tau, ps):
                  q_sb = qsbp.tile([128, 4, 64], bf16, tag="qsb")
                  nc.vector.tensor_copy(q_sb, ps[:, :, 0:64])
                  qt_ps = ps_tp.tile([64, 512], bf16, tag="ktps")
                  for r in range(4):
                      nc.tensor.transpose(
                          qt_ps[:, r * 128:(r + 1) * 128],
                          q_sb[:, r, :],
                          ident,
                      )
                  qt = qtgp.tile([64, 512], bf16, tag="qtg")
                  nc.vector.tensor_copy(qt, qt_ps)
                  qt_tiles[tau] = qt

              _pstate = {}

              def kv_units(gi):
                  """5 closures projecting kv group gi."""
                  def mk_kv(idx):
                      def f():
                          if idx == 0:
                              _pstate[gi] = ps_kvp.tile(
                                  [128, 4, 128], f32, tag="pskv",
                                  name="pskv",
                              )
                          proj_kv_chunk(gi, idx, _pstate[gi])
                      return f

                  return [mk_kv(i) for i in range(4)] + [
                      lambda: finish_kv(gi, _pstate.pop(gi))
                  ]

              def q_units(tau):
                  """5 closures projecting + transposing q tile tau."""
                  key = ("q", tau)

                  def mk_q(r):
                      def f():
                          if r == 0:
                              _pstate[key] = ps_kvp.tile(
                                  [128, 4, 128], f32, tag="pskv",
                                  name="psq",
                              )
                          proj_q(tau, r, _pstate[key])
                      return f

                  return [mk_q(r) for r in range(4)] + [
                      lambda: finish_q(tau, _pstate.pop(key))
                  ]

              # ---- attention ----------------------------------------
              def attend(tau, sched):
                  """A(tau), software-pipelined two pairs deep; `sched`
                  maps slot index -> closures (deadline-spread projection
                  work for this and the next tau, filling PE idle during
                  ACT-bound stretches)."""
                  qt = qt_tiles.pop(tau)
                  npairs = 4 * tau + 4
                  ps_av = ps_avp.tile([128, 4, 65], f32, tag="psav")
                  pend = {}

                  def emit_scores(pr):
                      c = 2 * pr
                      diag = pr >= 4 * tau
                      r0 = pr - 4 * tau if diag else 0
                      m = 512 - 128 * r0
                      ps = ps_sp.tile([128, 1024], f32, tag="pss")
                      # even chunk right-aligned bank0, odd left bank1
                      nc.tensor.matmul(
                          ps[:, 512 - m:512],
                          lhsT=ktg[c // 4][:, (c % 4) * 128:
                                           (c % 4 + 1) * 128],
                          rhs=qt[:, 128 * r0:512],
                          start=True, stop=True,
                      )
                      c1 = c + 1
                      nc.tensor.matmul(
                          ps[:, 512:512 + m],
                          lhsT=ktg[c1 // 4][:, (c1 % 4) * 128:
                                            (c1 % 4 + 1) * 128],
                          rhs=qt[:, 128 * r0:512],
                          start=True, stop=True,
                      )
                      p = probsp.tile([128, 1024], bf16, tag="p")
                      nc.scalar.activation(
                          p[:, 0:2 * m], ps[:, 512 - m:512 + m],
                          mybir.ActivationFunctionType.Exp, scale=0.125,
                      )
                      pend[pr] = (p, m, r0, diag)

                  def emit_av(pr):
                      p, m, r0, diag = pend.pop(pr)
                      if diag:
                          nc.vector.tensor_mul(
                              p[:, 0:128], p[:, 0:128], mask_sb[:, 0:128]
                          )
                          nc.vector.tensor_mul(
                              p[:, m:m + 128], p[:, m:m + 128],
                              mask_sb[:, 128:256],
                          )
                      for half in (0, 1):
                          ch = 2 * pr + half
                          off = half * m
                          vslice = kvg[ch // 4][:, ch % 4, 64:129]
                          for r in range(r0, 4):
                              nc.tensor.matmul(
                                  ps_av[:, r, :],
                                  lhsT=p[:, off + (r - r0) * 128:
                                         off + (r - r0 + 1) * 128],
                                  rhs=vslice,
                                  # one start per psum tile: the bank-wide
                                  # pending-zero mark must not be re-armed
                                  # by later region starts
                                  start=(ch == 0 and r == 0),
                                  stop=(ch == 8 * tau + 2 * r + 1),
                                  skip_group_check=True,
                              )

                  def emit_out(r):
                      # region r final after its diag pair: drain it now so
                      # the output DMA overlaps the remaining pairs
                      o_sb = osbp.tile([128, 65], f32, tag="osb")
                      nc.vector.tensor_copy(o_sb, ps_av[:, r, :])
                      eng = nc.scalar if tau < 2 else nc.sync
                      eng.dma_start(
                          out=out[512 * tau + 128 * r:
                                  512 * tau + 128 * (r + 1), :],
                          in_=o_sb,
                      )

                  # 2-deep software pipeline: scores of pairs i+1, i+2 are
                  # issued before AV of pair i so PE never heads-of-line
                  # blocks on the exp latency
                  depth = 2
                  for pr in range(npairs + depth):
                      for u in sched.get(pr, ()):
                          u()
                      if pr < npairs:
                          emit_scores(pr)
                      if pr >= depth:
                          emit_av(pr - depth)
                          if pr - depth >= 4 * tau:
                              emit_out(pr - depth - 4 * tau)
                  for u in sched.get("end", ()):
                      u()

              # ---- schedule -----------------------------------------
              # upfront DMA issue, need-ordered, spread over the three
              # DMA-capable queues (SP/ACT on HWDGE, gpsimd on SWDGE).
              # ACT gets only early ones (its SEQ issues exps from ~9us).
              rr = [nc.sync, nc.scalar, nc.gpsimd]
              for j in (0, 1):          # fine cols 0:1024
                  for cc in range(NCC):
                      dma_xt("f", j, cc, rr[(j * NCC + cc) % 3])
              nc.gpsimd.dma_start(out=wq_sb, in_=wq[:, :])
              nc.gpsimd.dma_start(out=mask_sb, in_=masks[:, :])
              for cc in range(NCC):     # big cols 1024:2304
                  # scalar (ACT) must be free before exps start (~8us):
                  # it gets only one; sync/gpsimd split the rest
                  eng = (nc.scalar if cc == 0
                         else nc.sync if cc < 5 else nc.gpsimd)
                  dma_xt("b", 0, cc, eng)
              for cc in range(NCC):     # big cols 2304:3584
                  dma_xt("b", 1, cc, nc.sync if cc % 2 else nc.gpsimd)
              for cc in range(NCC):     # fine cols 3584:4096
                  dma_xt("f", 7, cc, nc.sync if cc % 2 else nc.gpsimd)

              def spread(m, units, s0, s1):
                  # distribute units evenly over slots [s0, s1)
                  n = len(units)
                  span = max(1, s1 - s0)
                  for i, u in enumerate(units):
                      s = min(s1 - 1, s0 + (i * span) // n)
                      m.setdefault(s, []).append(u)

              # A(tau) pair pr touches kv chunks 2pr,2pr+1, i.e. kv group
              # g first at pair 2g; with the 2-deep pipeline its scores
              # are emitted at slot 2g. qt(tau) is needed at slot 0, so
              # q is projected during the previous tau.
              for u in kv_units(0) + q_units(0):
                  u()
              for tau in range(NT):
                  sched = {}
                  # slot 0 stays unit-free so each tau's first scores
                  # (and its exp) issue immediately at the boundary
                  if tau == 0:
                      spread(sched, kv_units(1), 1, 3)
                      spread(sched, q_units(1), 3, 6)
                  else:
                      spread(sched, kv_units(2 * tau), 1, 4 * tau)
                      spread(sched, kv_units(2 * tau + 1),
                             4 * tau, 4 * tau + 2)
                      if tau + 1 < NT:
                          spread(sched, q_units(tau + 1),
                                 4 * tau + 2, 4 * tau + 5)
                  attend(tau, sched)

    nc.compile()
    return nc


def _get_compiled():
    global _compiled
    if _compiled is None:
        _compiled = _build_nc()
    return _compiled


def _host_inputs(x, Wq, Wk, Wv):
    bf = ml_dtypes.bfloat16
    wq = np.concatenate(
        [Wq[c * 128:(c + 1) * 128] for c in range(NCC)], axis=1
    ).astype(bf)
    wkv_full = np.concatenate([Wk, Wv], axis=1)
    wkv = np.concatenate(
        [wkv_full[c * 128:(c + 1) * 128] for c in range(NCC)], axis=1
    ).astype(bf)

    j = np.arange(128)[:, None]
    i = np.arange(128)[None, :]
    tri = (j <= i).astype(np.float32)

    in_maps = []
    for core in range(N_CORES):
        b, p = core // 2, core % 2
        xb = x[b]                        # (T, C)
        if p == 1:
            xb = xb.reshape(NKV // 2, 2, 128, C)[:, ::-1].reshape(T, C)
        xtc = np.ascontiguousarray(xb.T, dtype=bf)
        m2 = np.full((128, 128), float(p), np.float32)
        mask = np.concatenate([tri, m2], axis=1).astype(bf)
        in_maps.append({"xt": xtc, "wq": wq, "wkv": wkv, "masks": mask})
    return in_maps


def kernel(x, Wq, Wk, Wv):
    from concourse.bass_utils import run_bass_kernel_spmd

    nc = _get_compiled()
    in_maps = _host_inputs(x, Wq, Wk, Wv)
    res = run_bass_kernel_spmd(nc, in_maps, core_ids=list(range(N_CORES)))

    out_full = np.empty((B, T, D), dtype=np.float32)
    for core in range(N_CORES):
        b, p = core // 2, core % 2
        acc = res.results[core]["out"]          # (2048, 65) f32
        o = acc[:, :D] / acc[:, D:D + 1]
        o = o.reshape(16, 128, D)
        for u in range(16):
            n = 2 * u + p
            out_full[b, 128 * n:128 * (n + 1), :] = o[u]
    return out_full


# revision 35
# speedup vs baseline: 1.0038x; 1.0009x over previous
"""Causal single-head attention  B=4, T=4096, C=1024, D=64  on 8 TRN2 cores.

Sharding: 2 cores per batch; queries split by 128-col chunk parity
(core p owns natural chunks {2u+p}) for causal balance. The parity is
handled purely by data: p=1 cores get xt with adjacent 128-col chunks
swapped, plus an all-ones "mask2" (p=0: all-zeros) for the one
program-order-future-but-naturally-past chunk per query chunk.

Per core (bf16 matmuls, fp32 accum), all in program chunk order:
  k|v proj  [seq,D]: out[kv,128] = xt_c.T @ [Wk|Wv]_c   (M=128/pass)
  q proj    [seq,D]: out[q,64]   = xt_g.T @ Wq_c        (M=64/pass)
  kT,qT via PE transpose (D=64 rows needed on partitions for scores)
  scoresT   = kT_chunk.T @ qT    (kv on partitions, q free)
  probs     = exp(0.125 * scoresT)  one ACT op per chunk-PAIR
              (two adjacent PSUM banks, right/left aligned)
  masks     : diag chunk = shared lower-tri; next chunk = mask2 (0/1)
  AV flipped: out[q,65] += probsT_slice.T @ [v|1]       (M=65/pass)
  host: out[b, owned chunks] = (acc[:, :64] / acc[:, 64:65])
"""

import sys

sys.path.insert(0, "/opt/trn_rl_repo")

import numpy as np
import ml_dtypes

B, T, C, D = 4, 4096, 1024, 64
NCC = C // 128          # 8 contraction chunks
NKV = T // 128          # 32 kv chunks (program order)
NT = 4                  # q tiles (4 q-chunks each, 512 cols)
N_CORES = 8

_compiled = None


def _build_nc(loop_n=None):
    import contextlib
    import concourse.bass as bass
    import concourse.bacc as bacc
    import concourse.mybir as mybir
    from concourse.tile import TileContext
    from concourse.masks import make_identity

    f32 = mybir.dt.float32
    bf16 = mybir.dt.bfloat16

    nc = bacc.Bacc("TRN2", target_bir_lowering=False, debug=False)
    xt = nc.dram_tensor("xt", (C, T), bf16, kind="ExternalInput")
    wq = nc.dram_tensor("wq", (128, NCC * D), bf16, kind="ExternalInput")
    wkv = nc.dram_tensor("wkv", (128, NCC * 2 * D), bf16, kind="ExternalInput")
    masks = nc.dram_tensor("masks", (128, 256), bf16, kind="ExternalInput")
    out = nc.dram_tensor("out", (T // 2, D + 1), f32, kind="ExternalOutput")

    with TileContext(nc) as tc:
        with (
            tc.tile_pool(name="const", bufs=1) as constp,
            tc.tile_pool(name="xtf", bufs=24) as xtfp,
            tc.tile_pool(name="xtb", bufs=16) as xtbp,
            tc.tile_pool(name="kvg", bufs=8) as kvgp,
            tc.tile_pool(name="ktg", bufs=8) as ktgp,
            tc.tile_pool(name="qsb", bufs=3) as qsbp,
            tc.tile_pool(name="qtg", bufs=2) as qtgp,
            tc.tile_pool(name="probs", bufs=6) as probsp,
            tc.tile_pool(name="osb", bufs=6) as osbp,
            tc.tile_pool(name="ps_s", bufs=2, space="PSUM") as ps_sp,
            tc.tile_pool(name="ps_kv", bufs=2, space="PSUM") as ps_kvp,
            tc.tile_pool(name="ps_t", bufs=1, space="PSUM") as ps_tp,
            tc.tile_pool(name="ps_av", bufs=1, space="PSUM") as ps_avp,
        ):
            ident = constp.tile([128, 128], bf16, tag="ident")
            make_identity(nc, ident)

            wq_sb = constp.tile([128, NCC * D], bf16, tag="wq")
            wkv_sb = constp.tile([128, NCC * 2 * D], bf16, tag="wkv")
            mask_sb = constp.tile([128, 256], bf16, tag="masks")
            # wkv gates the first projection: it goes first on sync; wq
            # and masks are needed later and are issued after the early
            # x tiles (see the DMA schedule below)
            nc.sync.dma_start(out=wkv_sb, in_=wkv[:, :])

            loop_cm = (
                tc.For_i(0, loop_n, 1) if loop_n else contextlib.nullcontext()
            )
            with loop_cm:
              xtiles = {}      # (cc, kind, j) -> tile
              kvg = {}         # kv group gi -> [128, 4, 130] bf16
              ktg = {}         # kv group gi -> [64, 512] bf16
              qt_tiles = {}    # tau -> [64, 512] bf16

              # xt tiling: fine [128,512] for cols 0:1024 and 3584:4096,
              # big [128,1280] for cols 1024:3584. Issue queues are round-
              # robined so no single sequencer bottlenecks DMA issue.
              def dma_xt(kind, j, cc, eng):
                  if kind == "f":
                      t = xtfp.tile([128, 512], bf16, tag="xf")
                      eng.dma_start(
                          out=t,
                          in_=xt[cc * 128:(cc + 1) * 128,
                                 j * 512:(j + 1) * 512],
                      )
                  else:
                      t = xtbp.tile([128, 1280], bf16, tag="xb")
                      eng.dma_start(
                          out=t,
                          in_=xt[cc * 128:(cc + 1) * 128,
                                 1024 + j * 1280:1024 + (j + 1) * 1280],
                      )
                  xtiles[(cc, kind, j)] = t

              def xcol(cc, c):
                  """AP for xt rows [128cc:+128], program cols [128c:+128)."""
                  col = c * 128
                  if col < 1024:
                      j = col // 512
                      return xtiles[(cc, "f", j)][:, col - 512 * j:
                                                  col - 512 * j + 128]
                  if col >= 3584:
                      return xtiles[(cc, "f", 7)][:, col - 3584:col - 3456]
                  g2 = (col - 1024) // 1280
                  o = col - 1024 - g2 * 1280
                  return xtiles[(cc, "b", g2)][:, o:o + 128]

              # ---- projection emission units -------------------------
              def proj_kv_chunk(gi, idx, ps):
                  c = gi * 4 + idx
                  for cc in range(NCC):
                      nc.tensor.matmul(
                          ps[:, idx, :],
                          lhsT=xcol(cc, c),
                          rhs=wkv_sb[:, cc * 128:(cc + 1) * 128],
                          start=(cc == 0),
                          stop=(cc == NCC - 1),
                      )

              def finish_kv(gi, ps):
                  g = kvgp.tile([128, 4, 130], bf16, tag="kvg")
                  nc.vector.tensor_copy(g[:, :, 0:128], ps)
                  nc.vector.memset(g[:, :, 128:129], 1.0)
                  kt_ps = ps_tp.tile([64, 512], bf16, tag="ktps")
                  for idx in range(4):
                      nc.tensor.transpose(
                          kt_ps[:, idx * 128:(idx + 1) * 128],
                          g[:, idx, 0:64],
                          ident,
                      )
                  kt = ktgp.tile([64, 512], bf16, tag="ktg")
                  nc.vector.tensor_copy(kt, kt_ps)
                  kvg[gi] = g
                  ktg[gi] = kt

              def proj_q(tau, r, ps):
                  g = 8 * tau + 2 * r
                  for cc in range(NCC):
                      nc.tensor.matmul(
                          ps[:, r, 0:64],
                          lhsT=xcol(cc, g),
                          rhs=wq_sb[:, cc * D:(cc + 1) * D],
                          start=(cc == 0),
                          stop=(cc == NCC - 1),
                      )

              def finish_q(tau, ps):
                  q_sb = qsbp.tile([128, 4, 64], bf16, tag="qsb")
                  nc.vector.tensor_copy(q_sb, ps[:, :, 0:64])
                  qt_ps = ps_tp.tile([64, 512], bf16, tag="ktps")
                  for r in range(4):
                      nc.tensor.transpose(
                          qt_ps[:, r * 128:(r + 1) * 128],
                          q_sb[:, r, :],
                          ident,
                      )
                  qt = qtgp.tile([64, 512], bf16, tag="qtg")
                  nc.vector.tensor_copy(qt, qt_ps)
                  qt_tiles[tau] = qt

              _pstate = {}

              def kv_units(gi):
                  """5 closures projecting kv group gi."""
                  def mk_kv(idx):
                      def f():
                          if idx == 0:
                              _pstate[gi] = ps_kvp.tile(
                                  [128, 4, 128], f32, tag="pskv",
                                  name="pskv",
                              )
                          proj_kv_chunk(gi, idx, _pstate[gi])
                      return f

                  return [mk_kv(i) for i in range(4)] + [
                      lambda: finish_kv(gi, _pstate.pop(gi))
                  ]

              def q_units(tau):
                  """5 closures projecting + transposing q tile tau."""
                  key = ("q", tau)

                  def mk_q(r):
                      def f():
                          if r == 0:
                              _pstate[key] = ps_kvp.tile(
                                  [128, 4, 128], f32, tag="pskv",
                                  name="psq",
                              )
                          proj_q(tau, r, _pstate[key])
                      return f

                  return [mk_q(r) for r in range(4)] + [
                      lambda: finish_q(tau, _pstate.pop(key))
                  ]

              # ---- attention ----------------------------------------
              def attend(tau, sched):
                  """A(tau), software-pipelined two pairs deep; `sched`
                  maps slot index -> closures (deadline-spread projection
                  work for this and the next tau, filling PE idle during
                  ACT-bound stretches)."""
                  qt = qt_tiles.pop(tau)
                  npairs = 4 * tau + 4
                  ps_av = ps_avp.tile([128, 4, 65], f32, tag="psav")
                  pend = {}

                  def emit_scores(pr):
                      c = 2 * pr
                      diag = pr >= 4 * tau
                      r0 = pr - 4 * tau if diag else 0
                      m = 512 - 128 * r0
                      ps = ps_sp.tile([128, 1024], f32, tag="pss")
                      # even chunk right-aligned bank0, odd left bank1
                      nc.tensor.matmul(
                          ps[:, 512 - m:512],
                          lhsT=ktg[c // 4][:, (c % 4) * 128:
                                           (c % 4 + 1) * 128],
                          rhs=qt[:, 128 * r0:512],
                          start=True, stop=True,
                      )
                      c1 = c + 1
                      nc.tensor.matmul(
                          ps[:, 512:512 + m],
                          lhsT=ktg[c1 // 4][:, (c1 % 4) * 128:
                                            (c1 % 4 + 1) * 128],
                          rhs=qt[:, 128 * r0:512],
                          start=True, stop=True,
                      )
                      p = probsp.tile([128, 1024], bf16, tag="p")
                      nc.scalar.activation(
                          p[:, 0:2 * m], ps[:, 512 - m:512 + m],
                          mybir.ActivationFunctionType.Exp, scale=0.125,
                      )
                      pend[pr] = (p, m, r0, diag)

                  def emit_av(pr):
                      p, m, r0, diag = pend.pop(pr)
                      if diag:
                          nc.vector.tensor_mul(
                              p[:, 0:128], p[:, 0:128], mask_sb[:, 0:128]
                          )
                          nc.vector.tensor_mul(
                              p[:, m:m + 128], p[:, m:m + 128],
                              mask_sb[:, 128:256],
                          )
                      for half in (0, 1):
                          ch = 2 * pr + half
                          off = half * m
                          vslice = kvg[ch // 4][:, ch % 4, 64:129]
                          for r in range(r0, 4):
                              nc.tensor.matmul(
                                  ps_av[:, r, :],
                                  lhsT=p[:, off + (r - r0) * 128:
                                         off + (r - r0 + 1) * 128],
                                  rhs=vslice,
                                  # one start per psum tile: the bank-wide
                                  # pending-zero mark must not be re-armed
                                  # by later region starts
                                  start=(ch == 0 and r == 0),
                                  stop=(ch == 8 * tau + 2 * r + 1),
                                  skip_group_check=True,
                              )

                  def emit_out(r):
                      # region r final after its diag pair: drain it now so
                      # the output DMA overlaps the remaining pairs
                      o_sb = osbp.tile([128, 65], f32, tag="osb")
                      nc.vector.tensor_copy(o_sb, ps_av[:, r, :])
                      eng = nc.scalar if tau < 2 else nc.sync
                      eng.dma_start(
                          out=out[512 * tau + 128 * r:
                                  512 * tau + 128 * (r + 1), :],
                          in_=o_sb,
                      )

                  # 2-deep software pipeline: scores of pairs i+1, i+2 are
                  # issued before AV of pair i so PE never heads-of-line
                  # blocks on the exp latency
                  depth = 2
                  for pr in range(npairs + depth):
                      for u in sched.get(pr, ()):
                          u()
                      if pr < npairs:
                          emit_scores(pr)
                      if pr >= depth:
                          emit_av(pr - depth)
                          if pr - depth >= 4 * tau:
                              emit_out(pr - depth - 4 * tau)
                  for u in sched.get("end", ()):
                      u()

              # ---- schedule -----------------------------------------
              # upfront DMA issue, need-ordered, spread over the three
              # DMA-capable queues (SP/ACT on HWDGE, gpsimd on SWDGE).
              # ACT gets only early ones (its SEQ issues exps from ~9us).
              rr = [nc.sync, nc.scalar, nc.gpsimd]
              for j in (0, 1):          # fine cols 0:1024
                  for cc in range(NCC):
                      dma_xt("f", j, cc, rr[(j * NCC + cc) % 3])
              nc.gpsimd.dma_start(out=wq_sb, in_=wq[:, :])
              nc.gpsimd.dma_start(out=mask_sb, in_=masks[:, :])
              for cc in range(NCC):     # big cols 1024:2304
                  # scalar (ACT) must be free before exps start (~8us):
                  # it gets only one; sync/gpsimd split the rest
                  eng = (nc.scalar if cc == 0
                         else nc.sync if cc < 5 else nc.gpsimd)
                  dma_xt("b", 0, cc, eng)
              for cc in range(NCC):     # big cols 2304:3584
                  dma_xt("b", 1, cc, nc.sync if cc % 2 else nc.gpsimd)
              for cc in range(NCC):     # fine cols 3584:4096
                  dma_xt("f", 7, cc, nc.sync if cc % 2 else nc.gpsimd)

              def spread(m, units, s0, s1):
                  # distribute units evenly over slots [s0, s1)
                  n = len(units)
                  span = max(1, s1 - s0)
                  for i, u in enumerate(units):
                      s = min(s1 - 1, s0 + (i * span) // n)
                      m.setdefault(s, []).append(u)

              # A(tau) pair pr touches kv chunks 2pr,2pr+1, i.e. kv group
              # g first at pair 2g; with the 2-deep pipeline its scores
              # are emitted at slot 2g. qt(tau) is needed at slot 0, so
              # q is projected during the previous tau.
              for u in kv_units(0) + q_units(0):
                  u()
              for tau in range(NT):
                  sched = {}
                  # slot 0 stays unit-free so each tau's first scores
                  # (and its exp) issue immediately at the boundary
                  if tau == 0:
                      spread(sched, kv_units(1), 1, 3)
                      spread(sched, q_units(1), 3, 6)
                  else:
                      spread(sched, kv_units(2 * tau), 1, 4 * tau)
                      spread(sched, kv_units(2 * tau + 1),
                             4 * tau, 4 * tau + 2)
                      if tau + 1 < NT:
                          spread(sched, q_units(tau + 1),
                                 4 * tau + 2, 4 * tau + 5)
                  attend(tau, sched)

    nc.compile()
    return nc


def _get_compiled():
    global _compiled
    if _compiled is None:
        _compiled = _build_nc()
    return _compiled


def _host_inputs(x, Wq, Wk, Wv):
    bf = ml_dtypes.bfloat16
    wq = np.concatenate(
        [Wq[c * 128:(c + 1) * 128] for c in range(NCC)], axis=1
    ).astype(bf)
    wkv_full = np.concatenate([Wk, Wv], axis=1)
    wkv = np.concatenate(
        [wkv_full[c * 128:(c + 1) * 128] for c in range(NCC)], axis=1
    ).astype(bf)

    j = np.arange(128)[:, None]
    i = np.arange(128)[None, :]
    tri = (j <= i).astype(np.float32)

    in_maps = []
    for core in range(N_CORES):
        b, p = core // 2, core % 2
        xb = x[b]                        # (T, C)
        if p == 1:
            xb = xb.reshape(NKV // 2, 2, 128, C)[:, ::-1].reshape(T, C)
        xtc = np.ascontiguousarray(xb.T, dtype=bf)
        m2 = np.full((128, 128), float(p), np.float32)
        mask = np.concatenate([tri, m2], axis=1).astype(bf)
        in_maps.append({"xt": xtc, "wq": wq, "wkv": wkv, "masks": mask})
    return in_maps


def kernel(x, Wq, Wk, Wv):
    from concourse.bass_utils import run_bass_kernel_spmd

    nc = _get_compiled()
    in_maps = _host_inputs(x, Wq, Wk, Wv)
    res = run_bass_kernel_spmd(nc, in_maps, core_ids=list(range(N_CORES)))

    out_full = np.empty((B, T, D), dtype=np.float32)
    for core in range(N_CORES):
        b, p = core // 2, core % 2
        acc = res.results[core]["out"]          # (2048, 65) f32
        o = acc[:, :D] / acc[:, D:D + 1]
        o = o.reshape(16, 128, D)
        for u in range(16):
            n = 2 * u + p
            out_full[b, 128 * n:128 * (n + 1), :] = o[u]
    return out_full
